# revision 25
# baseline (speedup 1.0000x reference)
"""Trainium2 Bass kernel for nn_BetterAttendCompareAggregate.

Math (per batch b, with q_b = q[:, b, :] [L, D], p_b = p[:, b, :] [L, D]):
    att_q = q_b @ WF^T ; att_p = p_b @ WF^T
    e = att_q @ att_p^T = q_b @ M @ p_b^T,      M = WF^T @ WF  (symmetric)
    sj = masked_softmax(e, m, axis=j), si = masked_softmax(e^T, m^T, axis=l)
    beta = sj @ p_b ; alpha = si @ q_b
    v1 = [q_b, beta] @ WG^T ; v2 = [p_b, alpha] @ WG^T
    out = [sum_l v1, sum_l v2] @ WH^T

Everything after the softmaxes is linear, so with
    g1 = WG^T @ WH[0,:D], g2 = WG^T @ WH[0,D:]   (each [2D])
    g1a, g1b = g1[:D], g1[D:] ; g2a, g2b = g2[:D], g2[D:]
    cj[j] = sum_l sj[l,j] ; ci[l] = sum_j si[j,l]
the output collapses to
    out[b] = sum_l q_l.g1a + sum_j cj[j]*(p_j.g1b)
           + sum_j p_j.g2a + sum_l ci[l]*(q_l.g2b)

Device work per batch: A = M @ q_b^T (one [1024x1024]x[1024x256] matmul),
e = A^T vs p^T both ways (two [256x1024]x[1024x256] matmuls), masked-softmax
statistics (row max / exp / row sum), colsum rows via 1-column matmuls, and
tiny G-dot rows. B=64 is sharded 8 batches per core (pure data parallel).

Host prep: M, g-vectors (float64 folding of the constant weights), and
d-major layouts of q/p so no on-chip transposes are needed.
"""

import os

import numpy as np

from concourse import bacc, mybir, tile
from concourse.bass_utils import run_bass_kernel_spmd

P = 128
D = 1024
L = 256
B = 64
NCORES = 8
NB = B // NCORES      # batches per core
KC = D // P           # contraction chunks
MC = D // P           # output chunks of A
LC = L // P           # chunks of L
NPAIR = NB // 2
EPS = 1e-6
F32 = mybir.dt.float32
NEG_HUGE = -3.4e38

# matmul streaming dtype: float32r runs at full PE rate (1 cyc/row for
# free-dim >= 256) with near-fp32 accuracy; float32 is exact but 4x slower.
MM_DT = mybir.dt.float32r


def _mm(ap):
    return ap


def _body(tc, qT, pT, Mt, Gq, Gp, pm_rows, qm_rows, qmT, pmT, out):
    nc = tc.nc
    AX = mybir.AxisListType.X
    OP = mybir.AluOpType

    with (
        tc.tile_pool(name="singles", bufs=1) as singles,
        tc.tile_pool(name="io", bufs=2) as io,
        tc.tile_pool(name="workA", bufs=2) as workA,
        tc.tile_pool(name="soft", bufs=3) as soft,
        tc.tile_pool(name="ex_pool", bufs=4) as ex_pool,
        tc.tile_pool(name="small", bufs=4) as small,
        tc.tile_pool(name="tail", bufs=4) as tailp,
        tc.tile_pool(name="ps_big", bufs=2, space="PSUM") as ps_big,
        tc.tile_pool(name="ps_e", bufs=3, space="PSUM") as ps_e,
        tc.tile_pool(name="ps_small", bufs=3, space="PSUM") as ps_small,
    ):
        # ---- constants (M split per k-chunk so matmuls start early) ----
        M_sb = singles.tile([P, KC, D], MM_DT)
        for k in range(KC):
            nc.gpsimd.dma_start(M_sb[:, k], Mt[k * P:(k + 1) * P, :])
        Gq_sb = singles.tile([P, KC, 33], MM_DT)
        nc.sync.dma_start(Gq_sb[:], Gq.rearrange("(ko p) g -> p ko g", p=P))
        Gp_sb = singles.tile([P, KC, 33], MM_DT)
        nc.sync.dma_start(Gp_sb[:], Gp.rearrange("(ko p) g -> p ko g", p=P))
        qmT_sb = singles.tile([P, LC, NB], F32)
        nc.sync.dma_start(qmT_sb[:], qmT.rearrange("(c p) b -> p c b", p=P))
        pmT_sb = singles.tile([P, LC, NB], F32)
        nc.sync.dma_start(pmT_sb[:], pmT.rearrange("(c p) b -> p c b", p=P))
        out_rowA = singles.tile([1, NB], F32)
        out_rowB = singles.tile([33, NB], F32)

        def softmax_stats(ps_chunks, row_mask2, rep_mask, ex_sb, tag, b):
            """Masked-softmax statistics for one orientation of e.

            ps_chunks: list of LC PSUM tiles [P, L] holding e rows.
            row_mask2: [P, LC] row-mask AP (per-partition scalar per chunk).
            rep_mask:  [P, L] column mask replicated across partitions.
            ex_sb:     [P, LC, L] out: exp(t - mx) * colmask.
            Returns rec [P, LC] = rowmask / (rowsum + eps).
            """
            den = small.tile([P, LC], F32, tag=f"den_{tag}", name=f"den_{tag}{b}")
            for c in range(LC):
                t_sb = soft.tile([P, L], F32, tag=f"t_{tag}", name=f"t_{tag}{b}{c}")
                # t = e * rowmask  (ACT: PSUM -> SBUF with per-partition scale)
                nc.scalar.mul(t_sb[:], ps_chunks[c][:], row_mask2[:, c:c + 1])
                # t = t * colmask   (matches ref: stats taken over x*m)
                nc.vector.tensor_tensor(t_sb[:], t_sb[:], rep_mask[:], OP.mult)
                negmx = small.tile([P, 1], F32, tag="negmx", name=f"ngm_{tag}{b}{c}")
                nc.vector.tensor_reduce(
                    negmx[:], t_sb[:], axis=AX, op=OP.max, negate=True
                )
                # t = exp(t - mx)
                nc.scalar.activation(
                    t_sb[:], t_sb[:], mybir.ActivationFunctionType.Exp,
                    bias=negmx[:], scale=1.0,
                )
                # ex = t * colmask ; den = rowsum(ex)
                nc.vector.tensor_tensor(ex_sb[:, c], t_sb[:], rep_mask[:], OP.mult)
                nc.vector.tensor_reduce(den[:, c:c + 1], ex_sb[:, c], axis=AX,
                                        op=OP.add)
            # rec = rowmask / (den + eps)   (batched over both chunks)
            nc.vector.tensor_scalar_add(den[:], den[:], EPS)
            rec = small.tile([P, LC], MM_DT, tag=f"rec_{tag}", name=f"rec_{tag}{b}")
            with nc.allow_low_precision(reason="float32r is ~fp32"):
                nc.vector.reciprocal(rec[:], den[:])
            nc.vector.tensor_tensor(rec[:], rec[:], row_mask2, OP.mult)
            return rec

        def emit_tail(state):
            """Colsum matmuls + final combine for a finished pair.

            Deferred until after the next pair's A matmuls so the PE never
            stalls waiting for the softmax chain.
            """
            for (b, exj, exi, recq, recp, qg_sb, pg_sb, i) in state:
                ps_cj = ps_small.tile([1, L], F32, tag="row", name=f"pscj{b}")
                for c in range(LC):
                    nc.tensor.matmul(
                        ps_cj[:], recq[:, c:c + 1], exj[:, c],
                        start=(c == 0), stop=(c == LC - 1),
                    )
                cj_sb = tailp.tile([1, L], F32, tag="cj", name=f"cj{b}")
                nc.vector.tensor_copy(cj_sb[:], ps_cj[:])
                ps_ci = ps_small.tile([1, L], F32, tag="row", name=f"psci{b}")
                for c in range(LC):
                    nc.tensor.matmul(
                        ps_ci[:], recp[:, c:c + 1], exi[:, c],
                        start=(c == 0), stop=(c == LC - 1),
                    )
                ci_sb = tailp.tile([1, L], F32, tag="ci", name=f"ci{b}")
                nc.vector.tensor_copy(ci_sb[:], ps_ci[:])

                # out[b] = [sum(pg_w*cj) + sum(qg_w*ci)]          (partition 0)
                #        + [sum(pg_plain) + sum(qg_plain)]           (partition 32)
                s0 = small.tile([1, 2], F32, tag="s0", name=f"s0_{b}")
                tp = small.tile([1, L], F32, tag="tscr", name=f"tp{b}")
                nc.vector.tensor_tensor(tp[:], pg_sb[0:1, i], cj_sb[:], OP.mult)
                nc.vector.tensor_reduce(out=s0[:, 0:1], in_=tp[:], axis=AX,
                                        op=OP.add)
                tq = small.tile([1, L], F32, tag="tscr", name=f"tq{b}")
                nc.vector.tensor_tensor(tq[:], qg_sb[0:1, i], ci_sb[:], OP.mult)
                nc.vector.tensor_reduce(out=s0[:, 1:2], in_=tq[:], axis=AX,
                                        op=OP.add)
                nc.vector.tensor_reduce(out=out_rowA[0:1, b:b + 1], in_=s0[:],
                                        axis=AX, op=OP.add)
                s32 = small.tile([33, 2], F32, tag="s32", name=f"s32_{b}")
                nc.vector.tensor_reduce(out=s32[32:33, 0:1], in_=pg_sb[32:33, i],
                                        axis=AX, op=OP.add)
                nc.vector.tensor_reduce(out=s32[32:33, 1:2], in_=qg_sb[32:33, i],
                                        axis=AX, op=OP.add)
                nc.vector.tensor_reduce(out=out_rowB[32:33, b:b + 1],
                                        in_=s32[32:33, :], axis=AX, op=OP.add)

        pending = None
        for g in range(NPAIR):
            q_sb = io.tile([P, KC, 2, L], MM_DT, tag="q", name=f"q_sb{g}")
            for h in range(2):
                nc.sync.dma_start(
                    q_sb[:, 4 * h:4 * h + 4],
                    qT[4 * h * P:(4 * h + 4) * P, 2 * g:2 * g + 2, :]
                    .rearrange("(ko p) b l -> p ko b l", p=P),
                )
            p_sb = io.tile([P, KC, 2, L], MM_DT, tag="p", name=f"p_sb{g}")
            for h in range(2):
                nc.sync.dma_start(
                    p_sb[:, 4 * h:4 * h + 4],
                    pT[4 * h * P:(4 * h + 4) * P, 2 * g:2 * g + 2, :]
                    .rearrange("(ko p) b l -> p ko b l", p=P),
                )

            # ---- A[d', i, l] = sum_d M[d, d'] * q[d, i, l]  (pair-batched) ----
            A_sb = workA.tile([P, MC, 2, L], MM_DT, tag="A", name=f"A_sb{g}")
            for m in range(MC):
                psA = ps_big.tile([P, 2, L], F32, tag="psA", name=f"psA{g}_{m}")
                for k in range(KC):
                    nc.tensor.matmul(
                        psA[:], M_sb[:, k, m * P:(m + 1) * P], q_sb[:, k],
                        start=(k == 0), stop=(k == KC - 1),
                    )
                # split PSUM->SBUF copies across Vector and Scalar engines
                if m % 2 == 0:
                    nc.vector.tensor_copy(A_sb[:, m], psA[:])
                else:
                    nc.scalar.copy(A_sb[:, m], psA[:])

            # ---- previous pair's colsums/finals (PE stays warm on A) ----
            if pending is not None:
                emit_tail(pending)
                pending = None

            # ---- G-dot rows: one matmul pass per side; rows 0 and 32 ----
            gdots = []
            for G_sb, x_sb, nm in ((Gq_sb, q_sb, "qg"), (Gp_sb, p_sb, "pg")):
                ps_g = ps_small.tile([33, 2, L], F32, tag="row",
                                     name=f"ps{nm}{g}")
                for k in range(KC):
                    nc.tensor.matmul(
                        ps_g[:], G_sb[:, k], x_sb[:, k],
                        start=(k == 0), stop=(k == KC - 1),
                    )
                g_sb = tailp.tile([33, 2, L], F32, tag=f"{nm}", name=f"{nm}{g}")
                nc.vector.tensor_copy(g_sb[:], ps_g[:])
                gdots.append(g_sb)
            qg_sb, pg_sb = gdots

            state = []
            for i in range(2):
                b = 2 * g + i

                # column masks replicated across partitions
                pm_rep = soft.tile([P, L], F32, tag="pm_rep", name=f"pmr{b}")
                nc.sync.dma_start(pm_rep[:],
                                  pm_rows[b:b + 1, :].to_broadcast((P, L)))
                qm_rep = soft.tile([P, L], F32, tag="qm_rep", name=f"qmr{b}")
                nc.sync.dma_start(qm_rep[:],
                                  qm_rows[b:b + 1, :].to_broadcast((P, L)))

                # ---- e chunks:  e[l, j] = sum_d' A[d', l] p[d', j] ----
                psE = []
                for c in range(LC):
                    ps = ps_e.tile([P, L], F32, tag="e", name=f"psE{b}_{c}")
                    for k in range(KC):
                        nc.tensor.matmul(
                            ps[:], A_sb[:, k, i, c * P:(c + 1) * P],
                            p_sb[:, k, i],
                            start=(k == 0), stop=(k == KC - 1),
                        )
                    psE.append(ps)
                psET = []
                for c in range(LC):
                    ps = ps_big.tile([P, L], F32, tag="psA", name=f"psET{b}_{c}")
                    for k in range(KC):
                        nc.tensor.matmul(
                            ps[:], p_sb[:, k, i, c * P:(c + 1) * P],
                            A_sb[:, k, i],
                            start=(k == 0), stop=(k == KC - 1),
                        )
                    psET.append(ps)

                # ---- masked softmax statistics ----
                exj = ex_pool.tile([P, LC, L], MM_DT, tag="exj", name=f"exj{b}")
                recq = softmax_stats(psE, qmT_sb[:, :, b], pm_rep, exj, "j", b)
                exi = ex_pool.tile([P, LC, L], MM_DT, tag="exi", name=f"exi{b}")
                recp = softmax_stats(psET, pmT_sb[:, :, b], qm_rep, exi, "i", b)
                state.append((b, exj, exi, recq, recp, qg_sb, pg_sb, i))
            pending = state

        emit_tail(pending)
        nc.sync.dma_start(out[0:1, :], out_rowA[:])
        nc.sync.dma_start(out[1:2, :], out_rowB[32:33, :])


_PROGRAM = None


def build_program():
    nc = bacc.Bacc(
        "TRN2", target_bir_lowering=False, debug=False, num_devices=NCORES
    )
    qT = nc.dram_tensor("qT", [D, NB, L], MM_DT, kind="ExternalInput").ap()
    pT = nc.dram_tensor("pT", [D, NB, L], MM_DT, kind="ExternalInput").ap()
    Mt = nc.dram_tensor("M", [D, D], MM_DT, kind="ExternalInput").ap()
    Gq = nc.dram_tensor("Gq", [D, 33], MM_DT, kind="ExternalInput").ap()
    Gp = nc.dram_tensor("Gp", [D, 33], MM_DT, kind="ExternalInput").ap()
    pm_rows = nc.dram_tensor("pm_rows", [NB, L], F32, kind="ExternalInput").ap()
    qm_rows = nc.dram_tensor("qm_rows", [NB, L], F32, kind="ExternalInput").ap()
    qmT = nc.dram_tensor("qmT", [L, NB], F32, kind="ExternalInput").ap()
    pmT = nc.dram_tensor("pmT", [L, NB], F32, kind="ExternalInput").ap()
    out = nc.dram_tensor("out", [2, NB], F32, kind="ExternalOutput").ap()
    with tile.TileContext(nc) as tc:
        _body(tc, qT, pT, Mt, Gq, Gp, pm_rows, qm_rows, qmT, pmT, out)
    nc.compile()
    return nc


def get_program():
    global _PROGRAM
    if _PROGRAM is None:
        _PROGRAM = build_program()
    return _PROGRAM


def make_in_maps(q, p, qm, pm, WF, WG, WH):
    WF64 = WF.astype(np.float64)
    M = (WF64.T @ WF64).astype(np.float32)
    WGT = WG.astype(np.float64).T                     # [2D, D]
    g1 = WGT @ WH[0, :D].astype(np.float64)           # [2D]
    g2 = WGT @ WH[0, D:].astype(np.float64)
    # [D, 33]: col 0 = colsum-weighted vector, col 32 = plain-sum vector.
    # Cols 1-31 are zero padding so the two output rows land on partitions
    # 0 and 32 (the legal DVE base partitions); partition-0 ops handle the
    # weighted terms, partition-32 ops the plain sums, and the host adds
    # the two resulting output rows.
    Gq = np.zeros((D, 33), np.float32)
    Gq[:, 0] = g2[D:]; Gq[:, 32] = g1[:D]
    Gp = np.zeros((D, 33), np.float32)
    Gp[:, 0] = g1[D:]; Gp[:, 32] = g2[:D]
    in_maps = []
    for c in range(NCORES):
        bs = slice(c * NB, (c + 1) * NB)
        qT = np.ascontiguousarray(q[:, bs, :].transpose(2, 1, 0))  # [D, NB, L]
        pT = np.ascontiguousarray(p[:, bs, :].transpose(2, 1, 0))
        qmf = np.ascontiguousarray(qm[bs].astype(np.float32))      # [NB, L]
        pmf = np.ascontiguousarray(pm[bs].astype(np.float32))
        in_maps.append({
            "qT": qT, "pT": pT, "M": M, "Gq": Gq, "Gp": Gp,
            "pm_rows": pmf, "qm_rows": qmf,
            "qmT": np.ascontiguousarray(qmf.T),
            "pmT": np.ascontiguousarray(pmf.T),
        })
    return in_maps


def install_profile_hook():
    """Provide antenv.axon_hooks if the image lacks it (NTFF profiling)."""
    import sys
    import types

    try:
        from antenv.axon_hooks import get_axon_ntff_profile_hook  # noqa: F401
        return True
    except ImportError:
        pass
    try:
        from trn_agent_boot.trn_boot import _ntff_profile_via_ctypes

        hook = _ntff_profile_via_ctypes("/opt/axon/libaxon_pjrt.so")
        if hook is None:
            return False
        mod = types.ModuleType("antenv.axon_hooks")
        mod._hook = hook
        mod.get_axon_ntff_profile_hook = lambda: mod._hook

        def _set(h):
            mod._hook = h

        mod.set_axon_ntff_profile_hook = _set
        import antenv

        antenv.axon_hooks = mod
        sys.modules["antenv.axon_hooks"] = mod
        return True
    except Exception as e:  # pragma: no cover
        print(f"install_profile_hook failed: {e}")
        return False


def run(in_maps, trace=False, **kwargs):
    nc = get_program()
    if trace:
        install_profile_hook()
    return run_bass_kernel_spmd(
        nc, in_maps, core_ids=list(range(NCORES)), trace=trace, **kwargs
    )


def kernel(q, p, qm, pm, WF, WG, WH):
    in_maps = make_in_maps(
        np.asarray(q), np.asarray(p), np.asarray(qm), np.asarray(pm),
        np.asarray(WF), np.asarray(WG), np.asarray(WH),
    )
    res = run(in_maps, trace=False)
    return assemble_out(res)


def assemble_out(res):
    outs = []
    for c in range(NCORES):
        o = res.results[c]["out"]          # [2, NB]: partition-0 + partition-32 rows
        outs.append((o[0] + o[1]).reshape(NB, 1))
    return np.ascontiguousarray(np.concatenate(outs, axis=0).astype(np.float32))
